# revision 40
# baseline (speedup 1.0000x reference)
"""Trainium2 Bass kernel for nn_Decoder (LSTM decoder + additive attention + vocab head).

Sharding (8 NeuronCores, SPMD — identical program, per-core data):
  - LSTM h/c recurrence replicated on all cores, transposed layout (units on
    partitions, batch on free). Weight-stationary bf16 matmuls; x-projection
    precomputed via indirect-DMA embedding gather + PE transposes, chunked so
    step 0 starts ~8us in.  Gates reordered (i,f,o,g) so one tanh covers all
    three sigmoids.
  - Attention + s_t = tanh(Wc @ [ctx; h]) sharded over batch (4/core) via a
    host-side batch permutation (own batches always in slots 0-3).
  - s gathered with 4 AllGathers (8-step blocks) overlapping the recurrence;
    vocab projection sharded over V (4000 rows/core), bias folded into the
    PSUM eviction (tensor-tensor add vs a broadcast bias tile), bf16 output.
Only Tanh/Exp ACT funcs are used (single table set); sigmoid via tanh.
"""

import numpy as np
import ml_dtypes

V, E, H, LQ, B = 32000, 256, 512, 32, 32
NCORES = 8
VS = V // NCORES      # 4000
VCH = 500             # vocab chunk (fp32 psum bank = 512)
NVC = VS // VCH       # 8
NBLK = LQ // 4        # 8 four-step attention blocks
NAG = 2               # allgather rounds (16-step blocks; each costs ~55us
                      # of serialized CC time incl. inter-collective gap)

_cache = {}


def _build_program():
    import concourse.bass as bass
    import concourse.mybir as mybir
    import concourse.tile as tile
    from concourse import bacc

    f32 = mybir.dt.float32
    bf16 = mybir.dt.bfloat16
    i32 = mybir.dt.int32

    nc = bacc.Bacc("TRN2", target_bir_lowering=False, debug=False,
                   num_devices=NCORES)

    d = {}

    def inp(name, shape, dtype):
        d[name] = nc.dram_tensor(name, shape, dtype, kind="ExternalInput").ap()

    inp("emb", [V, E], bf16)
    inp("idx", [128, 8], i32)
    inp("whh", [128, 64 * 128], bf16)
    inp("wih", [128, 32 * 128], bf16)
    inp("wa", [128, 16 * 128], bf16)
    inp("wc", [128, 32 * 128], bf16)
    inp("ua", [128, 16 * 128], bf16)
    inp("bias_units", [128, 16], f32)
    inp("bias_kp", [128, 4], f32)
    inp("va", [128, 4], bf16)
    inp("vab", [1, 1], f32)
    inp("wcb", [128, 4], f32)
    inp("h0t", [128, 128], bf16)
    inp("encr", [128, 512], bf16)
    inp("enct", [128, 512], bf16)
    inp("mask", [1, 128], i32)
    inp("bdmask", [128, 4], f32)
    inp("wht", [512, VS], bf16)
    inp("whbb", [128, VS], bf16)
    d["out"] = nc.dram_tensor("out", [B, LQ, VS], bf16, kind="ExternalOutput").ap()
    d["ag_in"] = [nc.dram_tensor(f"agi{a}", [128, 256], bf16).ap()
                  for a in range(NAG)]
    d["ag_out"] = [nc.dram_tensor(f"ago{a}", [128 * NCORES, 256], bf16,
                                  addr_space="Shared").ap() for a in range(NAG)]
    d["agd_in"] = nc.dram_tensor("agdi", [128, 2], bf16).ap()
    d["agd_out"] = nc.dram_tensor("agdo", [128 * NCORES, 2], bf16,
                                  addr_space="Shared").ap()

    with tile.TileContext(nc) as tc:
        _emit(tc, bass, mybir, d)
    nc.compile()
    return nc


def _emit(tc, bass, mybir, d):
    from concourse.masks import make_identity

    f32 = mybir.dt.float32
    bf16 = mybir.dt.bfloat16
    i32 = mybir.dt.int32
    AF = mybir.ActivationFunctionType
    OP = mybir.AluOpType
    nc = tc.nc

    perm = tc.alloc_tile_pool(name="perm", bufs=1)
    est = tc.alloc_tile_pool(name="est", bufs=2)
    big = tc.alloc_tile_pool(name="big", bufs=1)
    psG = tc.alloc_tile_pool(name="psG", bufs=2, space="PSUM")
    psA = tc.alloc_tile_pool(name="psA", bufs=2, space="PSUM")
    psV = tc.alloc_tile_pool(name="psV", bufs=4, space="PSUM")
    outp = tc.alloc_tile_pool(name="outp", bufs=3)

    def pt(name, shape, dtype):
        return perm.tile(shape, dtype, tag=name, name=name)

    # ---- persistent SBUF ----------------------------------------------
    whh = pt("whh", [128, 64 * 128], bf16)   # lhsT tile (kt,mt) @ (kt*16+mt)*128
    wih = pt("wih", [128, 32 * 128], bf16)   # (kt*16+mt)*128, kt<2
    wa = pt("wa", [128, 16 * 128], bf16)     # (kt*4+mt)*128
    wc = pt("wc", [128, 32 * 128], bf16)     # (kt*4+mt)*128, kt<8
    ua = pt("ua", [128, 16 * 128], bf16)
    bias_u = pt("bias_u", [128, 16], f32)
    bias_kp = pt("bias_kp", [128, 4], f32)
    va_sb = pt("va_sb", [128, 4], bf16)
    vab_sb = pt("vab_sb", [1, 1], f32)
    wcb_sb = pt("wcb_sb", [128, 4], f32)
    idx_sb = pt("idx_sb", [128, 8], i32)
    encr = pt("encr", [128, 512], bf16)      # rows (b_own,l), cols u
    enct = pt("enct", [128, 512], bf16)       # cols (kt, (b_own,l))
    mask_i = pt("mask_i", [1, 128], i32)
    mask01 = pt("mask01", [1, 128], f32)     # (b_own, l)
    bdm = pt("bdm", [128, 4], f32)
    hist = pt("hist", [128, 33 * 128], bf16)  # col = slot*128 + kt*32 + b
    cT = pt("cT", [128, 128], f32)
    xpb = pt("xpb", [128, 16 * 1024], bf16)  # col = mt*1024 + t*32 + b
    xT = pt("xT", [128, 2 * 1024], bf16)     # col = kt*1024 + (t*32+b)
    kp = pt("kp", [128, 512], f32)           # col = mt*128 + b*32 + l
    sT = pt("sT", [128, 512], bf16)           # col = ublk*128 + a*16 + (b*4+t)
    wh_sb = pt("wh_sb", [128, 4 * VS], bf16)  # col = kt*VS + v
    whb_sb = pt("whb_sb", [128, VS], bf16)    # Wh_b broadcast to partitions
    ident = pt("ident", [128, 128], f32)
    ident_b = pt("ident_b", [128, 128], bf16)
    ones_b = pt("ones_b", [1, 128], bf16)
    sg = pt("sg", [128, NBLK * 4 * 128], bf16)  # col = a*512+kt*128+(j*16+b*4+t)

    hist_v = hist[:].rearrange("p (s k b) -> p s k b", s=33, k=4)
    xpb_v = xpb[:].rearrange("p (m t) -> p m t", m=16)
    kp_v = kp[:].rearrange("p (m b l) -> p m b l", m=4, b=4)
    sT_v = sT[:].rearrange("p (k a i) -> p k a i", k=4, a=NBLK)
    sg_v = sg[:].rearrange("p (a k i) -> p a k i", a=NBLK, k=4)
    sg_v5 = sg[:].rearrange("p (a k j i) -> p a k j i", a=NBLK, k=4, j=NCORES)

    # ---- load weights/constants ---------------------------------------
    # dummy collective input staged first so its warmup trigger fires ASAP
    dummy = est.tile([128, 2], bf16, tag="dummy", name="dummy")
    nc.gpsimd.memset(dummy[:], 0.0)
    nc.sync.dma_start(d["agd_in"][:], dummy[:])
    # sync queue: recurrence-critical first
    nc.sync.dma_start(idx_sb[:], d["idx"][:])
    nc.sync.dma_start(hist[:, 0:128], d["h0t"][:])
    # bulk weights stay on sync so the scalar engine's instruction stream is
    # never blocked behind big DMA dispatches (ACTs share that stream)
    for dst, src in ((whh, "whh"), (mask_i, "mask"), (bdm, "bdmask"),
                     (wc, "wc"), (whb_sb, "whbb")):
        nc.sync.dma_start(dst[:], d[src][:])
    for kt in range(4):
        nc.sync.dma_start(wh_sb[:, kt * VS:(kt + 1) * VS],
                          d["wht"][kt * 128:(kt + 1) * 128, :])
    # scalar queue: small x-projection + attention weights only
    for dst, src in ((wih, "wih"), (bias_u, "bias_units"), (enct, "enct"),
                     (ua, "ua"), (bias_kp, "bias_kp"), (wa, "wa"),
                     (va_sb, "va"), (vab_sb, "vab"), (encr, "encr"),
                     (wcb_sb, "wcb")):
        nc.scalar.dma_start(dst[:], d[src][:])

    make_identity(nc, ident[:])
    make_identity(nc, ident_b[:])
    nc.gpsimd.memset(ones_b[:], 1.0)
    nc.gpsimd.memset(cT[:], 0.0)

    mask_f = est.tile([1, 128], f32, tag="mf", name="mask_f")
    nc.vector.tensor_copy(out=mask_f[:], in_=mask_i[:])
    nc.vector.tensor_scalar(out=mask01[:], in0=mask_f[:], scalar1=0.0,
                            scalar2=None, op0=OP.not_equal)

    # ---- embedding gather + transpose (per 4-step chunk c) -------------
    def gather(c):
        xrow = est.tile([128, 256], bf16, tag="xrow", name="xrow")
        nc.gpsimd.indirect_dma_start(
            out=xrow[:], out_offset=None, in_=d["emb"][:],
            in_offset=bass.IndirectOffsetOnAxis(ap=idx_sb[:, c:c + 1], axis=0))
        for kt in range(2):
            tp = psA.tile([128, 128], bf16, tag="a", name="tpx")
            nc.tensor.transpose(tp[:], xrow[:, kt * 128:(kt + 1) * 128],
                                ident_b[:, 0:128])
            seg = xT[:, kt * 1024 + c * 128: kt * 1024 + (c + 1) * 128]
            nc.vector.tensor_copy(out=seg, in_=tp[:])

    # ---- x-projection quarter c=0 (steps 0..3), all 16 mt --------------
    def xp_quarter(c):
        for g in range(4):
            ps = psV.tile([128, 512], f32, tag="v", name="xp_ps")
            for mi in range(4):
                mt = g * 4 + mi
                for kt in range(2):
                    nc.tensor.matmul(
                        ps[:, mi * 128:(mi + 1) * 128],
                        wih[:, (kt * 16 + mt) * 128:(kt * 16 + mt + 1) * 128],
                        xT[:, kt * 1024 + c * 128: kt * 1024 + (c + 1) * 128],
                        start=(kt == 0), stop=(kt == 1), skip_group_check=True)
            for mi in range(4):
                mt = g * 4 + mi
                dst = xpb_v[:, mt, c * 128:(c + 1) * 128]
                nc.vector.tensor_scalar(
                    out=dst, in0=ps[:, mi * 128:(mi + 1) * 128],
                    scalar1=bias_u[:, mt:mt + 1], scalar2=None, op0=OP.add)

    # ---- x-projection steps 4..15 (c=1..3), 4 mt per call --------------
    def xp_rest1(g):
        for mi in range(4):
            mt = g * 4 + mi
            ps = psV.tile([128, 384], f32, tag="v", name="xp1_ps")
            for c in range(1, 4):
                for kt in range(2):
                    nc.tensor.matmul(
                        ps[:, (c - 1) * 128:c * 128],
                        wih[:, (kt * 16 + mt) * 128:(kt * 16 + mt + 1) * 128],
                        xT[:, kt * 1024 + c * 128: kt * 1024 + (c + 1) * 128],
                        start=(kt == 0), stop=(kt == 1), skip_group_check=True)
            dst = xpb[:, mt * 1024 + 128: mt * 1024 + 512]
            nc.vector.tensor_scalar(out=dst, in0=ps[:],
                                    scalar1=bias_u[:, mt:mt + 1],
                                    scalar2=None, op0=OP.add)

    # ---- x-projection half-2 (steps 16..31), 4 mt per call -------------
    def xp_half2(g):
        for mi in range(4):
            mt = g * 4 + mi
            ps = psV.tile([128, 512], f32, tag="v", name="xp2_ps")
            for kt in range(2):
                nc.tensor.matmul(
                    ps[:], wih[:, (kt * 16 + mt) * 128:(kt * 16 + mt + 1) * 128],
                    xT[:, kt * 1024 + 512: kt * 1024 + 1024],
                    start=(kt == 0), stop=(kt == 1))
            dst = xpb[:, mt * 1024 + 512: mt * 1024 + 1024]
            nc.vector.tensor_scalar(out=dst, in0=ps[:],
                                    scalar1=bias_u[:, mt:mt + 1],
                                    scalar2=None, op0=OP.add)

    # ---- key projection kp = Ua @ enc^T + (Ua_b + Wa_b), 2 mt per call -
    def kp_part(h):
        for mt in (2 * h, 2 * h + 1):
            ps = psA.tile([128, 128], f32, tag="a", name="kp_ps")
            for kt in range(4):
                nc.tensor.matmul(
                    ps[:], ua[:, (kt * 4 + mt) * 128:(kt * 4 + mt + 1) * 128],
                    enct[:, kt * 128:(kt + 1) * 128],
                    start=(kt == 0), stop=(kt == 3))
            nc.vector.tensor_scalar(out=kp[:, mt * 128:(mt + 1) * 128],
                                    in0=ps[:], scalar1=bias_kp[:, mt:mt + 1],
                                    scalar2=None, op0=OP.add)

    # ==== per-step bodies ==============================================
    def lstm_step(t):
        G = psG.tile([128, 512], f32, tag="g", name="G")
        nc.tensor.matmul(G[:], ident_b[:], xpb_v[:, :, t * 32:(t + 1) * 32],
                         start=True, stop=False, skip_group_check=True)
        for mt in range(16):
            for kt in range(4):
                nc.tensor.matmul(
                    G[:, mt * 32:(mt + 1) * 32],
                    whh[:, (kt * 16 + mt) * 128:(kt * 16 + mt + 1) * 128],
                    hist_v[:, t, kt, :], start=False, stop=(kt == 3),
                    skip_group_check=True)
        # gates reordered on host: i,f,o | g  (sigmoid block contiguous)
        tifo = est.tile([128, 384], f32, tag="tifo", name="tifo")
        tg = est.tile([128, 128], f32, tag="tg", name="tg")
        nc.scalar.activation(tifo[:], G[:, 0:384], AF.Tanh, scale=0.5)
        nc.scalar.activation(tg[:], G[:, 384:512], AF.Tanh)
        sig = est.tile([128, 384], f32, tag="sig", name="sig")
        nc.vector.tensor_scalar(out=sig[:], in0=tifo[:], scalar1=0.5,
                                scalar2=0.5, op0=OP.mult, op1=OP.add)
        ig = est.tile([128, 128], f32, tag="ig", name="ig")
        fc = est.tile([128, 128], f32, tag="fc", name="fc")
        nc.vector.tensor_tensor(out=ig[:], in0=sig[:, 0:128], in1=tg[:],
                                op=OP.mult)
        nc.gpsimd.tensor_tensor(out=fc[:], in0=sig[:, 128:256], in1=cT[:],
                                op=OP.mult)
        nc.vector.tensor_tensor(out=cT[:], in0=ig[:], in1=fc[:], op=OP.add)
        tct = est.tile([128, 128], f32, tag="tct", name="tct")
        nc.scalar.activation(tct[:], cT[:], AF.Tanh)
        nc.vector.tensor_tensor(out=hist[:, (t + 1) * 128:(t + 2) * 128],
                                in0=sig[:, 256:384], in1=tct[:], op=OP.mult)

    attn_state = {}

    def attention_a(a):
        t0 = 4 * a
        qp = psA.tile([128, 4, 4, 4], f32, tag="a", name="qp")  # (mt, t, b)
        for mt in range(4):
            for kt in range(4):
                nc.tensor.matmul(
                    qp[:, mt, :, :],
                    wa[:, (kt * 4 + mt) * 128:(kt * 4 + mt + 1) * 128],
                    hist_v[:, t0 + 1:t0 + 5, kt, 0:4],
                    start=(kt == 0), stop=(kt == 3))
        tin = big.tile([128, 4, 4, 4, 32], f32, tag="tin", name="tin")
        tnh = big.tile([128, 2048], bf16, tag="tnh", name="tnh")
        for mt in range(4):
            nc.vector.tensor_tensor(
                out=tin[:, mt],
                in0=qp[:, mt].unsqueeze(3).to_broadcast([128, 4, 4, 32]),
                in1=kp_v[:, mt].unsqueeze(1).to_broadcast([128, 4, 4, 32]),
                op=OP.add)
            nc.scalar.activation(tnh[:, mt * 512:(mt + 1) * 512],
                                 tin[:, mt].rearrange("p b c l -> p (b c l)"),
                                 AF.Tanh)
        attn_state[a] = dict(tnh=tnh)

    def attention_b(a):
        tnh = attn_state[a]["tnh"]
        sc = psA.tile([1, 512], f32, tag="a", name="sc")
        for ub in range(4):
            nc.tensor.matmul(sc[:], va_sb[:, ub:ub + 1],
                             tnh[:, ub * 512:(ub + 1) * 512],
                             start=(ub == 0), stop=(ub == 3))
        esc = est.tile([1, 512], f32, tag="esc", name="esc")
        nc.scalar.activation(esc[:], sc[:], AF.Exp, bias=vab_sb[:, 0:1])
        escm = est.tile([1, 4, 4, 32], f32, tag="escm", name="escm")
        nc.vector.tensor_tensor(
            out=escm[:],
            in0=esc[:].rearrange("p (t b l) -> p t b l", t=4, b=4),
            in1=mask01[:].rearrange("p (b l) -> p b l", b=4).unsqueeze(1)
            .to_broadcast([1, 4, 4, 32]), op=OP.mult)
        den = est.tile([1, 16], f32, tag="den", name="den")
        nc.vector.tensor_reduce(out=den[:],
                                in_=escm[:].rearrange("p t b l -> p (t b) l"),
                                axis=mybir.AxisListType.X, op=OP.add)
        rden = est.tile([1, 16], f32, tag="rden", name="rden")
        nc.vector.reciprocal(rden[:], den[:])
        attn = est.tile([1, 4, 4, 32], f32, tag="attn", name="attn")
        nc.vector.tensor_tensor(
            out=attn[:], in0=escm[:],
            in1=rden[:].rearrange("p (t b) -> p t b", t=4).unsqueeze(3)
            .to_broadcast([1, 4, 4, 32]), op=OP.mult)
        tep = psA.tile([128, 4], f32, tag="a", name="tep")
        for tt in range(4):
            nc.tensor.transpose(tep[:, tt:tt + 1],
                                attn[:, tt].rearrange("p b l -> p (b l)"),
                                ident[0:1, 0:1])
        tes = est.tile([128, 4], f32, tag="tes", name="tes")
        nc.vector.tensor_copy(out=tes[:], in_=tep[:])
        # block-diag weights A[(b,l), (b',t)] = attn^T[(b,l), t] * [b'==b]
        abig = est.tile([128, 4, 4], bf16, tag="abig", name="abig")
        nc.vector.tensor_tensor(
            out=abig[:], in0=bdm[:].unsqueeze(2).to_broadcast([128, 4, 4]),
            in1=tes[:].unsqueeze(1).to_broadcast([128, 4, 4]), op=OP.mult)
        # ctx computed directly transposed: ctxT[u,(b,t)] = enc^T @ abig
        ctp = psA.tile([128, 64], f32, tag="a", name="ctp")
        for ub in range(4):
            nc.tensor.matmul(ctp[:, ub * 16:(ub + 1) * 16],
                             encr[:, ub * 128:(ub + 1) * 128],
                             abig[:].rearrange("p b t -> p (b t)"),
                             start=True, stop=True, skip_group_check=True)
        cxt = est.tile([128, 64], bf16, tag="cxt", name="cxt")
        nc.vector.tensor_copy(out=cxt[:], in_=ctp[:])
        attn_state[a]["cxt"] = cxt

    def attention_c(a):
        t0 = 4 * a
        cxt = attn_state[a]["cxt"]
        sp = psA.tile([128, 64], f32, tag="a", name="sp")
        for mt in range(4):
            for kt in range(8):
                rhs = (cxt[:, kt * 16:(kt + 1) * 16] if kt < 4
                       else hist_v[:, t0 + 1:t0 + 5, kt - 4, 0:4]
                       .rearrange("p t b -> p b t"))
                nc.tensor.matmul(
                    sp[:, mt * 16:(mt + 1) * 16],
                    wc[:, (kt * 4 + mt) * 128:(kt * 4 + mt + 1) * 128],
                    rhs, start=(kt == 0), stop=(kt == 7),
                    skip_group_check=True)
        for mt in range(4):
            nc.scalar.activation(
                sT_v[:, mt, a, :], sp[:, mt * 16:(mt + 1) * 16], AF.Tanh,
                bias=wcb_sb[:, mt:mt + 1])

    def ag_round(A):
        # sT sub-blocks 4A..4A+3 -> ag_in[A]; small DMA on the scalar queue
        # (the sT ACTs it waits for run on scalar right before it)
        nc.scalar.dma_start(
            d["ag_in"][A][:].rearrange("p (k q i) -> p k q i", k=4, q=4),
            sT_v[:, :, 4 * A:4 * A + 4, :])
        nc.gpsimd.collective_compute(
            "AllGather", OP.bypass, replica_groups=[list(range(NCORES))],
            ins=[d["ag_in"][A][:]], outs=[d["ag_out"][A][:]])

    def sg_read(a, t):
        # gate: tiny write into the sg region, dependent on the current LSTM
        # step, so the scheduler cannot hoist the sg DMA (and the vocab
        # matmuls behind it) to a point where the AllGather is still in
        # flight at runtime — the cost model underestimates mesh latency.
        # sg read-back rides the sync queue: nothing latency-critical behind.
        nc.vector.tensor_copy(out=sg_v5[:, a, 0, 0, 0:1],
                              in_=hist[:, (t + 1) * 128:(t + 1) * 128 + 1])
        ago_v = d["ag_out"][a // 4].rearrange("(j p) (k q i) -> q p k j i",
                                              j=NCORES, k=4, q=4)
        nc.sync.dma_start(sg_v5[:, a], ago_v[a % 4])

    out_v = d["out"].rearrange("(j bl) (a tl) v -> a (j bl) tl v",
                               j=NCORES, a=NBLK)

    def vocab_half(a, h, tail=False):
        for vc in range(4 * h, 4 * h + 4):
            # in-loop: even chunks get bias via a cheap PE matmul + Scalar
            # copy-eviction; odd chunks (and the whole tail) fold the bias
            # into a Vector tensor-tensor eviction (GPSIMD can't read PSUM).
            use_v = tail or (vc % 2 == 1)
            ps = psV.tile([128, VCH], f32, tag="v", name="vps")
            if not use_v:
                nc.tensor.matmul(ps[:], ones_b[:, 0:128],
                                 whb_sb[0:1, vc * VCH:(vc + 1) * VCH],
                                 start=True, stop=False, skip_group_check=True)
            for kt in range(4):
                nc.tensor.matmul(
                    ps[:], sg_v[:, a, kt],
                    wh_sb[:, kt * VS + vc * VCH: kt * VS + (vc + 1) * VCH],
                    start=(use_v and kt == 0), stop=(kt == 3),
                    skip_group_check=True)
            ob = outp.tile([128, VCH], bf16, tag="ob", name="ob")
            if use_v:
                nc.vector.tensor_tensor(
                    out=ob[:], in0=ps[:],
                    in1=whb_sb[:, vc * VCH:(vc + 1) * VCH], op=OP.add)
            else:
                nc.scalar.activation(ob[:], ps[:], AF.Copy)
            q = nc.sync if vc % 2 == 0 else nc.scalar
            q.dma_start(out_v[a, :, :, vc * VCH:(vc + 1) * VCH], ob[:])

    # ==== schedule ======================================================
    gather(0)
    # dummy collective fired ASAP to absorb the ~60us first-collective
    # warmup on the CC core before AG(0) needs it
    nc.gpsimd.collective_compute(
        "AllGather", OP.bypass, replica_groups=[list(range(NCORES))],
        ins=[d["agd_in"][:]], outs=[d["agd_out"][:]])
    xp_quarter(0)

    for t in range(LQ):
        with tc.high_priority():
            lstm_step(t)
        if t == 0:
            gather(1)
            gather(2)
            gather(3)
            xp_rest1(0)
        elif t == 1:
            xp_rest1(1)
            gather(4)
            gather(5)
        elif t == 2:
            xp_rest1(2)
            gather(6)
            gather(7)
            kp_part(0)
            kp_part(1)
        elif t == 3:
            xp_rest1(3)
        if t >= 3 and (t - 3) % 4 == 0:
            attention_a((t - 3) // 4)
        if t >= 4 and (t - 4) % 4 == 0:
            attention_b((t - 4) // 4)
        if t >= 5 and (t - 5) % 4 == 0:
            a = (t - 5) // 4
            attention_c(a)
            if a == 3:
                ag_round(0)
        if t in (4, 5, 6, 7):
            xp_half2(t - 4)

    # all vocab work is post-loop: AG(0)'s mesh finishes around when the
    # recurrence does, so in-loop vocab could only stall the PE queue
    attention_b(7)
    attention_c(7)
    ag_round(1)
    for a in range(8):
        sg_read(a, LQ - 1)
        for h in range(2):
            vocab_half(a, h)

    for pool in (outp, psV, psA, psG, big, est, perm):
        pool.release()


# ======================================================================
# host side
# ======================================================================

def _bf16(x):
    return np.ascontiguousarray(np.asarray(x, np.float32).astype(ml_dtypes.bfloat16))


def _tiles(wT, ktn, mtn):
    """[K, M] -> [128, ktn*mtn*128]; tile (kt,mt) at col (kt*mtn+mt)*128."""
    K, M = wT.shape
    assert K == ktn * 128 and M == mtn * 128
    t = wT.reshape(ktn, 128, mtn, 128).transpose(1, 0, 2, 3)
    return np.ascontiguousarray(t.reshape(128, ktn * mtn * 128))


# gate reorder: [i, f, g, o] -> [i, f, o, g]
_REORD = np.r_[0:512, 512:1024, 1536:2048, 1024:1536]


def kernel(src_padding_mask, enc_hidden_states, enc_last_hidden_state,
           tgt_batch, sos_idx, emb, W_ih, W_hh, b_ih, b_hh, Wa_w, Wa_b,
           Ua_w, Ua_b, va_w, va_b, Wc_w, Wc_b, Wh_w, Wh_b):
    import concourse.bass_utils as bass_utils

    if "nc" not in _cache:
        _cache["nc"] = _build_program()
    nc = _cache["nc"]

    f32 = np.float32
    emb_b = _bf16(emb)
    enc = np.asarray(enc_hidden_states, f32)
    h0 = np.asarray(enc_last_hidden_state, f32)[0]
    mask = np.asarray(src_padding_mask, np.int32)

    ids = np.empty((LQ, B), np.int64)
    ids[0, :] = int(sos_idx)
    ids[1:, :] = np.asarray(tgt_batch)[:, :-1].T

    whh_t = _bf16(_tiles(np.asarray(W_hh, f32)[_REORD].T, 4, 16))
    wih_t = _bf16(_tiles(np.asarray(W_ih, f32)[_REORD].T, 2, 16))
    wa_t = _bf16(_tiles(np.asarray(Wa_w, f32).T, 4, 4))
    wc_t = _bf16(_tiles(np.asarray(Wc_w, f32).T, 8, 4))
    ua_t = _bf16(_tiles(np.asarray(Ua_w, f32).T, 4, 4))
    bias_units = np.ascontiguousarray(
        (np.asarray(b_ih, f32) + np.asarray(b_hh, f32))[_REORD]
        .reshape(16, 128).T)
    bkp = np.ascontiguousarray(
        (np.asarray(Ua_b, f32) + np.asarray(Wa_b, f32)).reshape(4, 128).T)
    va_c = _bf16(np.asarray(va_w, f32)[0].reshape(4, 128).T)
    wcb = np.ascontiguousarray(np.asarray(Wc_b, f32).reshape(4, 128).T)
    bdm = np.zeros((128, 4), f32)
    for p in range(128):
        bdm[p, p // 32] = 1.0
    vab = np.full((1, 1), float(np.asarray(va_b, f32).reshape(-1)[0]), f32)
    WhT = _bf16(np.asarray(Wh_w, f32).T)
    Whb = np.asarray(Wh_b, f32)

    in_maps = []
    for j in range(NCORES):
        own = np.arange(4 * j, 4 * j + 4)
        permb = np.concatenate([own, np.setdiff1d(np.arange(B), own)])
        ids_p = ids[:, permb]
        idx = np.ascontiguousarray(
            ids_p.reshape(LQ * B).astype(np.int32).reshape(8, 128).T)
        h0p = h0[permb]
        h0t = np.zeros((128, 128), f32)
        for kt in range(4):
            h0t[:, kt * 32:(kt + 1) * 32] = h0p[:, kt * 128:(kt + 1) * 128].T
        enc_own = enc[own]                                   # [4, 32, 512]
        encr_j = _bf16(enc_own.reshape(128, 512))
        enctl = _bf16(
            enc_own.reshape(128, 4, 128).transpose(2, 1, 0).reshape(128, 512))
        whb_j = _bf16(np.broadcast_to(Whb[j * VS:(j + 1) * VS], (128, VS)))
        in_maps.append({
            "emb": emb_b, "idx": idx, "whh": whh_t, "wih": wih_t, "wa": wa_t,
            "wc": wc_t, "ua": ua_t, "bias_units": bias_units, "bias_kp": bkp,
            "va": va_c, "vab": vab, "wcb": wcb, "h0t": _bf16(h0t),
            "encr": encr_j, "enct": enctl,
            "mask": np.ascontiguousarray(mask[own].reshape(1, 128)),
            "bdmask": bdm,
            "wht": np.ascontiguousarray(WhT[:, j * VS:(j + 1) * VS]),
            "whbb": whb_j,
        })

    res = bass_utils.run_bass_kernel_spmd(nc, in_maps, list(range(NCORES)))
    out = np.concatenate([res.results[jj]["out"] for jj in range(NCORES)],
                         axis=2)
    return np.ascontiguousarray(out.astype(np.float32))


if __name__ == "__main__":
    import reference
    inp = dict(reference.setup_inputs())
    got = kernel(**{k: (np.asarray(v) if hasattr(v, "shape") else v)
                    for k, v in inp.items()})
    print("out shape", got.shape, got.dtype)
